# revision 1
# baseline (speedup 1.0000x reference)
"""Multi-head causal attention on 8 Trainium2 cores.

Sharding: core = (batch b in 0..3, head-group g in 0..1). Each core computes
Q/K/V projections for its 8 heads of its batch, causal attention, and a
partial output projection (Wo row-split); host sums the two partials per
batch and transposes back.

Device layout notes:
  - All matmul inputs are float32r (full fp32 storage; PE rounds the
    mantissa, 4x faster than plain fp32 matmul, ~1.5e-4 relative error).
  - Scores are computed transposed (S^T[k, q]) so softmax probabilities feed
    the P@V matmul directly as the moving operand (no transposes anywhere).
  - Softmax denominator Z[q] comes from an all-ones stationary matmul that
    broadcasts the partition-dim sum to all 128 partitions.
  - exp() folds in the 1/sqrt(hd) scale; causal masking is an additive
    -1e30 DVE add into PSUM on the 4 diagonal k-tiles of each q-chunk.
  - DRAM spill layouts are chosen so every reload is one contiguous run per
    partition (DMA descriptor-count, i.e. sync-engine issue time, is the
    scarce resource, not bandwidth).
"""

import numpy as np

import concourse.bacc as bacc
import concourse.mybir as mybir
import concourse.tile as tile
from concourse.bass_utils import run_bass_kernel_spmd

B, T, D = 4, 2048, 2048
NH, HD = 16, 128
G = 8                       # heads per core
GD = G * HD                 # 1024, group channel width
P = 128
QC = 512                    # q-chunk (PSUM bank width in fp32)
NKT = T // P                # 16 k-tiles over the sequence
NDK = D // P                # 16 k-tiles over d_in
NQC = T // QC               # 4 q-chunks
SCALE = 1.0 / float(np.sqrt(HD))
NEG = -1.0e30

F32 = mybir.dt.float32
F32R = mybir.dt.float32r


def build_kernel():
    nc = bacc.Bacc("TRN2", target_bir_lowering=False, debug=False, num_devices=8,
                   dynamic_dma_scratch_size=2048)

    xT = nc.dram_tensor("xT", [D, T], F32R, kind="ExternalInput")
    # pre-tiled on host: wq/wk [head, p, ko, d], wv [dchunk, p, ko, c]
    wqT = nc.dram_tensor("wqT", [G, P, NDK, HD], F32R, kind="ExternalInput")
    wkT = nc.dram_tensor("wkT", [G, P, NDK, HD], F32R, kind="ExternalInput")
    wvT = nc.dram_tensor("wvT", [2, P, NDK, QC], F32R, kind="ExternalInput")
    woT = nc.dram_tensor("woT", [GD, D], F32R, kind="ExternalInput")
    # maskadd[:, 0:128] = triangle (masked iff i > col), [:, 128:384] = j3 mask
    maskadd = nc.dram_tensor("maskadd", [P, 384], F32, kind="ExternalInput")
    outT = nc.dram_tensor("outT", [D, T], F32, kind="ExternalOutput")

    # [d_in partition, k-tile, t]
    xT_t = xT.rearrange("(ko p) t -> p ko t", p=P)
    woT_t = woT.rearrange("(co p) n -> p co n", p=P)
    outT_t = outT.rearrange("(no p) t -> p no t", p=P)

    with tile.TileContext(nc) as tc:
        with tc.tile_pool(name="dram", bufs=1, space="DRAM") as dpool:
            # Q^T: [d-partition, chunk, head, q] -> chunk reload is contiguous
            qt_d = dpool.tile([P, NQC, G, QC], F32R)
            # K^T per head: [head, d-partition, t] -> prefix reload contiguous
            kt_d = dpool.tile([G, P, T], F32R)
            # V per head: [head, t%128, t-tile, hd] -> prefix reload contiguous
            v_d = dpool.tile([G, P, NKT, HD], F32R)

            # ---------------- Phase A: projections ----------------
            with (
                tc.tile_pool(name="xpool", bufs=1) as xpool,
                tc.tile_pool(name="wpool", bufs=2) as wpool,
                tc.tile_pool(name="wvpool", bufs=2) as wvpool,
                tc.tile_pool(name="outa", bufs=3) as outa,
                tc.tile_pool(name="psA", bufs=4, space="PSUM") as psA,
            ):
                # First weight tile before the bulk x^T load so PE starts early.
                w0_sb = wpool.tile([P, NDK, P], F32R, tag="wqk")
                nc.sync.dma_start(w0_sb, wqT[0])

                xt_sb = xpool.tile([P, NDK, T], F32R)    # 16 MB, resident
                for k in range(NDK):
                    nc.sync.dma_start(xt_sb[:, k], xT_t[:, k])

                # Q^T and K^T: for each head, out[d(128), t] = sum_k W^T[k, d] x^T[k, t]
                for wt, w_ap in ((0, wqT), (1, wkT)):
                    for h in range(G):
                        if wt == 0 and h == 0:
                            w_sb = w0_sb
                        else:
                            w_sb = wpool.tile([P, NDK, P], F32R, tag="wqk")
                            nc.sync.dma_start(w_sb, w_ap[h])
                        for c in range(NQC):
                            ps = psA.tile([P, QC], F32, tag="qk_ps")
                            for k in range(NDK):
                                nc.tensor.matmul(
                                    ps,
                                    w_sb[:, k],
                                    xt_sb[:, k, c * QC:(c + 1) * QC],
                                    start=(k == 0),
                                    stop=(k == NDK - 1),
                                )
                            o_sb = outa.tile([P, QC], F32R, tag="qk_out")
                            nc.scalar.copy(o_sb, ps)
                            if wt == 0:
                                nc.sync.dma_start(qt_d[:, c, h], o_sb)
                            else:
                                nc.sync.dma_start(
                                    kt_d[h, :, c * QC:(c + 1) * QC], o_sb
                                )

                # V: out[t(128), c(512)] = sum_k x^T[k, t] wvT[k, c]
                # t-tile outer so early rows of V (needed by attention chunk 0)
                # land in DRAM first.
                wv_sbs = []
                for dc in range(2):
                    wv_sb = wvpool.tile([P, NDK, QC], F32R, tag="wv")
                    nc.sync.dma_start(wv_sb, wvT[dc])
                    wv_sbs.append(wv_sb)
                for ts_ in range(NKT):
                    for dc in range(2):
                        ps = psA.tile([P, QC], F32, tag="v_ps")
                        for k in range(NDK):
                            nc.tensor.matmul(
                                ps,
                                xt_sb[:, k, ts_ * P:(ts_ + 1) * P],
                                wv_sbs[dc][:, k],
                                start=(k == 0),
                                stop=(k == NDK - 1),
                            )
                        o_sb = outa.tile([P, QC], F32R, tag="v_out")
                        nc.vector.tensor_copy(o_sb, ps)
                        # route tail writes off the sync queue so phase B's
                        # first loads aren't stuck behind them
                        weng = nc.scalar if ts_ >= NKT - 4 else nc.sync
                        weng.dma_start(
                            v_d[dc * 4:(dc + 1) * 4, :, ts_, :].rearrange(
                                "g p c -> p g c"),
                            o_sb.rearrange("p (g c) -> p g c", g=4),
                        )

            # ---------------- Phase B: attention + output projection ----------------
            with (
                tc.tile_pool(name="wopool", bufs=1) as wopool,
                tc.tile_pool(name="const", bufs=1) as constp,
                tc.tile_pool(name="qpool", bufs=4) as qpool,
                tc.tile_pool(name="kpool", bufs=4) as kpool,
                tc.tile_pool(name="vpool", bufs=4) as vpool,
                tc.tile_pool(name="ppool", bufs=4) as ppool,
                tc.tile_pool(name="cpool", bufs=1) as cpool,
                tc.tile_pool(name="zpool", bufs=2) as zpool,
                tc.tile_pool(name="opool", bufs=3) as opool,
                tc.tile_pool(name="psSO", bufs=4, space="PSUM") as psSO,
                tc.tile_pool(name="psZ", bufs=2, space="PSUM") as psZ,
                tc.tile_pool(name="psC", bufs=2, space="PSUM") as psC,
            ):
                ones_sb = constp.tile([P, P], F32R)
                nc.vector.memset(ones_sb.bitcast(F32), 1.0)
                mask_sb = constp.tile([P, 384], F32)
                nc.sync.dma_start(mask_sb, maskadd[:])
                wo_sb = wopool.tile([P, G, D], F32R)      # 8 MB, resident

                # Chunk pairs: K/V loaded once per head per pair (covers both
                # chunks), doubling per-head PE work per load.
                for half in range(NQC // 2):
                    chunks = (2 * half, 2 * half + 1)
                    n_kt_hi = 4 * (chunks[1] + 1)

                    ctx2_sb = cpool.tile([P, 2, G, QC], F32R)

                    for h in range(G):
                        k_sb = kpool.tile([P, NKT, P], F32R, tag="ksb")
                        nc.sync.dma_start(
                            k_sb[:, :n_kt_hi],
                            kt_d[h, :, : n_kt_hi * P].rearrange(
                                "p (ko q) -> p ko q", q=P),
                        )
                        v_sb = vpool.tile([P, NKT, HD], F32R, tag="vsb")
                        nc.sync.dma_start(v_sb[:, :n_kt_hi], v_d[h, :, :n_kt_hi, :])
                        q2_sb = qpool.tile([P, 2, QC], F32R, tag="qsb")
                        nc.sync.dma_start(q2_sb, qt_d[:, 2 * half:2 * half + 2, h])

                        for ci, c in enumerate(chunks):
                            n_kt = 4 * (c + 1)
                            z_ps = psZ.tile([P, QC], F32, tag="z_ps")
                            ctx_ps = psC.tile([P, QC], F32, tag="ctx_ps")

                            # full tiles first, diagonal tiles last: a
                            # diagonal exp waiting on its DVE mask-add is
                            # strict-FIFO head-of-line blocking for ACT, so
                            # keep mask-gated exps out of the stream's way.
                            kt_order = [kt for kt in range(n_kt) if kt < 4 * c]
                            kt_order += [kt for kt in range(n_kt) if kt >= 4 * c]
                            for ki, kt in enumerate(kt_order):
                                j = kt - 4 * c
                                # diagonal tiles: only columns >= s0 can be
                                # valid (keep free >= 256 so fp32r stays at
                                # 1 cyc/row)
                                s0 = 0 if j < 1 else min(j * P, 2 * P)
                                sl = slice(s0, QC)
                                s_ps = psSO.tile([P, QC], F32, tag="so_ps")
                                nc.tensor.matmul(
                                    s_ps[:, sl], k_sb[:, kt], q2_sb[:, ci, sl],
                                    start=True, stop=True,
                                )
                                if 0 <= j < 3:
                                    nc.vector.tensor_add(
                                        s_ps[:, j * P:(j + 1) * P],
                                        s_ps[:, j * P:(j + 1) * P],
                                        mask_sb[:, 0:P],
                                    )
                                elif j == 3:
                                    nc.vector.tensor_add(
                                        s_ps[:, 2 * P:QC],
                                        s_ps[:, 2 * P:QC],
                                        mask_sb[:, P:3 * P],
                                    )
                                p_sb = ppool.tile([P, QC], F32R, tag="p_sb")
                                nc.scalar.activation(
                                    p_sb[:, sl], s_ps[:, sl],
                                    mybir.ActivationFunctionType.Exp,
                                    scale=SCALE,
                                )
                                nc.tensor.matmul(
                                    z_ps[:, sl], ones_sb, p_sb[:, sl],
                                    start=(ki == 0), stop=(ki == n_kt - 1),
                                )
                                nc.tensor.matmul(
                                    ctx_ps[:, sl], v_sb[:, kt], p_sb[:, sl],
                                    start=(ki == 0), stop=(ki == n_kt - 1),
                                )

                            iz_sb = zpool.tile([P, QC], F32, tag="iz")
                            nc.vector.reciprocal_approx_fast(iz_sb, z_ps)
                            nc.vector.tensor_mul(ctx2_sb[:, ci, h], ctx_ps, iz_sb)

                        if half == 0:
                            # Interleave the 8 MB Wo load with pair-0 head
                            # DMAs so no single burst starves the PE's K/V/Q
                            # feeds.
                            nc.sync.dma_start(wo_sb[:, h], woT_t[:, h])

                    # out^T[n(128), q] = sum_c woT[c, n] ctx^T[c, q]
                    for ci, c in enumerate(chunks):
                        q_sl = slice(c * QC, (c + 1) * QC)
                        for nt in range(NDK):
                            o_ps = psSO.tile([P, QC], F32, tag="so_ps")
                            for hh in range(G):
                                nc.tensor.matmul(
                                    o_ps,
                                    wo_sb[:, hh, nt * P:(nt + 1) * P],
                                    ctx2_sb[:, ci, hh],
                                    start=(hh == 0),
                                    stop=(hh == G - 1),
                                )
                            o_sb = opool.tile([P, QC], F32, tag="o_sb")
                            nc.scalar.copy(o_sb, o_ps)
                            nc.sync.dma_start(outT_t[:, nt, q_sl], o_sb)

    nc.finalize()
    return nc


_NC = None


def _get_nc():
    global _NC
    if _NC is None:
        _NC = build_kernel()
    return _NC


def _make_masks():
    m = np.zeros((P, 384), dtype=np.float32)
    i = np.arange(P)[:, None]
    col = np.arange(P)[None, :]
    m[:, 0:P][i > col] = NEG                  # triangle for j in {0,1,2}
    m[:, P:2 * P] = NEG                       # j=3: cols 256..383 fully masked
    m[:, 2 * P:3 * P][i > col] = NEG          # j=3: cols 384..511 triangle
    return m


def kernel(x, Wq, Wk, Wv, Wo, _trace=False, _trace_kwargs=None):
    x = np.asarray(x, dtype=np.float32)
    Wq = np.asarray(Wq, dtype=np.float32)
    Wk = np.asarray(Wk, dtype=np.float32)
    Wv = np.asarray(Wv, dtype=np.float32)
    Wo = np.asarray(Wo, dtype=np.float32)

    nc = _get_nc()
    masks = _make_masks()

    # [d_out, d_in] -> [h, p, ko, dd] tiles per head-group chunk of 8 heads
    def tile_qk(W, g):
        wt = W.T[:, g * GD:(g + 1) * GD]              # [D, GD]
        return np.ascontiguousarray(
            wt.reshape(NDK, P, G, HD).transpose(2, 1, 0, 3))

    def tile_v(W, g):
        wt = W.T[:, g * GD:(g + 1) * GD]              # [D, GD]
        return np.ascontiguousarray(
            wt.reshape(NDK, P, 2, QC).transpose(2, 1, 0, 3))

    woT = np.ascontiguousarray(Wo.T)

    in_maps = []
    for core in range(8):
        b, g = divmod(core, 2)
        gs = slice(g * GD, (g + 1) * GD)
        in_maps.append({
            "xT": np.ascontiguousarray(x[b].T),
            "wqT": tile_qk(Wq, g),
            "wkT": tile_qk(Wk, g),
            "wvT": tile_v(Wv, g),
            "woT": np.ascontiguousarray(woT[gs, :]),
            "maskadd": masks,
        })

    kwargs = {}
    if _trace:
        kwargs.update(trace=True, **(_trace_kwargs or {}))
    res = run_bass_kernel_spmd(nc, in_maps, core_ids=list(range(8)), **kwargs)

    out = np.empty((B, T, D), dtype=np.float32)
    for b in range(B):
        acc = res.results[2 * b]["outT"] + res.results[2 * b + 1]["outT"]
        out[b] = acc.T
    if _trace:
        return out, res
    return out



# revision 7
# speedup vs baseline: 1.0646x; 1.0646x over previous
"""Multi-head causal attention on 8 Trainium2 cores.

Sharding: core = (batch b in 0..3, head-group g in 0..1). Each core computes
Q/K/V projections for its 8 heads of its batch, causal attention, and a
partial output projection (Wo row-split); host sums the two partials per
batch and transposes back.

Device layout notes (v2 — bf16 SBUF-resident):
  - All matmul inputs are bf16 (1 cyc/row on PE, same as fp32r, but half the
    SBUF footprint and no >=256 free-size constraint). PSUM accumulates fp32.
  - Q^T, K^T, V for all 8 heads stay resident in SBUF between the projection
    phase and attention (20 MB total in bf16) — no DRAM spill/reload at all.
  - x^T streams in k-slice order so the first projection accumulation runs
    at DMA pace instead of waiting for the full tensor.
  - Softmax denominator: exp tiles are pair-summed on DVE (bf16 2x mode)
    into an fp32 accumulator; one [128,128]-ones matmul per (head, chunk)
    broadcasts the partition-sum. This removes the per-k-tile ones-matmul
    from PE (~60 us/core).
  - Scores are computed transposed (S^T[k, q]) so softmax probabilities feed
    the P@V matmul directly as the moving operand.
  - Diagonal k-tiles compute only their valid column range (bf16 has no
    free-size penalty) and two diagonal tiles share one PSUM tile + one exp.
  - Half-0's output projection is interleaved into half-1's attention at
    head granularity so the softmax z-chain latency never stalls PE.
"""

import numpy as np
import ml_dtypes

import concourse.bacc as bacc
import concourse.mybir as mybir
import concourse.tile as tile
from concourse.bass_utils import run_bass_kernel_spmd

B, T, D = 4, 2048, 2048
NH, HD = 16, 128
G = 8                       # heads per core
GD = G * HD                 # 1024, group channel width
P = 128
QC = 512                    # q-chunk (PSUM bank width in fp32)
NKT = T // P                # 16 k-tiles over the sequence
NDK = D // P                # 16 k-tiles over d_in
NQC = T // QC               # 4 q-chunks
SCALE = 1.0 / float(np.sqrt(HD))
NEG = -1.0e30

F32 = mybir.dt.float32
F32R = mybir.dt.float32r
BF16 = mybir.dt.bfloat16
EXP = mybir.ActivationFunctionType.Exp


def build_kernel():
    nc = bacc.Bacc("TRN2", target_bir_lowering=False, debug=False, num_devices=8,
                   dynamic_dma_scratch_size=2048)

    xT = nc.dram_tensor("xT", [D, T], BF16, kind="ExternalInput")
    # pre-tiled on host: wq/wk [head, p, ko, d], wv [dchunk, p, ko, c]
    wqT = nc.dram_tensor("wqT", [G, P, NDK, HD], BF16, kind="ExternalInput")
    wkT = nc.dram_tensor("wkT", [G, P, NDK, HD], BF16, kind="ExternalInput")
    wvT = nc.dram_tensor("wvT", [2, P, NDK, QC], BF16, kind="ExternalInput")
    woT = nc.dram_tensor("woT", [GD, D], BF16, kind="ExternalInput")
    # triangle mask: NEG where partition (k) > column (q) within a 128 block
    maskadd = nc.dram_tensor("maskadd", [P, P], F32, kind="ExternalInput")
    outT = nc.dram_tensor("outT", [D, T], F32, kind="ExternalOutput")

    xT_t = xT.rearrange("(ko p) t -> p ko t", p=P)
    woT_t = woT.rearrange("(co p) n -> p co n", p=P)
    outT_t = outT.rearrange("(no p) t -> p no t", p=P)

    with tile.TileContext(nc) as tc:
        with (
            tc.tile_pool(name="const", bufs=1) as constp,
            tc.tile_pool(name="kvq", bufs=1) as kvqp,
        ):
            ones_sb = constp.tile([P, P], F32R)
            nc.vector.memset(ones_sb.bitcast(F32), 1.0)
            mask_sb = constp.tile([P, P], F32)
            nc.sync.dma_start(mask_sb, maskadd[:])

            # SBUF-resident per-head tensors (bf16)
            kt_sb = kvqp.tile([P, G, T], BF16)           # K^T per head
            qt_sb = kvqp.tile([P, G, T], BF16)           # Q^T per head
            vt_sb = kvqp.tile([P, NKT, G, HD], BF16)     # V per head

            # ---------------- Phase A: projections ----------------
            with (
                tc.tile_pool(name="xpool", bufs=1) as xpool,
                tc.tile_pool(name="wvpool", bufs=1) as wvp,
                tc.tile_pool(name="wqkpool", bufs=2) as wqkp,
                tc.tile_pool(name="psV", bufs=2, space="PSUM") as psV,
                tc.tile_pool(name="psQK", bufs=2, space="PSUM") as psQK,
            ):
                # first Q weight before the x stream so PE starts ASAP
                w_first = wqkp.tile([P, NDK, HD], BF16, tag="w")
                nc.scalar.dma_start(w_first, wqT[0])

                xt_sb = xpool.tile([P, NDK, T], BF16)    # 8 MB, resident
                xqueues = [nc.sync, nc.gpsimd]
                for k in range(NDK):
                    xqueues[k % 2].dma_start(xt_sb[:, k], xT_t[:, k])

                wv_sb = wvp.tile([P, 2, NDK, QC], BF16)
                for dc in range(2):
                    nc.scalar.dma_start(wv_sb[:, dc], wvT[dc])

                # Q^T and K^T: out[d(128), t] = sum_k W^T[k, d] x^T[k, t]
                # k ascends so the first head streams with the x DMA.
                for wt, w_ap in ((0, wqT), (1, wkT)):
                    for h in range(G):
                        if wt == 0 and h == 0:
                            w_sb = w_first
                        else:
                            w_sb = wqkp.tile([P, NDK, HD], BF16, tag="w")
                            nc.scalar.dma_start(w_sb, w_ap[h])
                        dst = qt_sb if wt == 0 else kt_sb
                        for c in range(NQC):
                            ps = psQK.tile([P, QC], F32, tag=f"qk{c % 2}")
                            for k in range(NDK):
                                nc.tensor.matmul(
                                    ps, w_sb[:, k],
                                    xt_sb[:, k, c * QC:(c + 1) * QC],
                                    start=(k == 0), stop=(k == NDK - 1),
                                )
                            nc.scalar.copy(dst[:, h, c * QC:(c + 1) * QC], ps)

                # V: out[t(128), c(512)] = sum_k x^T[k, t] wvT[k, c]
                for ts in range(NKT):
                    for dc in range(2):
                        ps = psV.tile([P, QC], F32, tag=f"v{dc}")
                        for k in range(NDK):
                            nc.tensor.matmul(
                                ps, xt_sb[:, k, ts * P:(ts + 1) * P],
                                wv_sb[:, dc, k],
                                start=(k == 0), stop=(k == NDK - 1),
                            )
                        nc.vector.tensor_copy(
                            vt_sb[:, ts, 4 * dc:4 * (dc + 1), :],
                            ps.rearrange("p (g c) -> p g c", g=4),
                        )

            # ---------------- Phase B: attention + output projection ----------------
            with (
                tc.tile_pool(name="wopool", bufs=1) as wop,
                tc.tile_pool(name="c2pool", bufs=1) as c2p,
                tc.tile_pool(name="ppool", bufs=4) as pp,
                tc.tile_pool(name="prpool", bufs=2) as prp,
                tc.tile_pool(name="accpool", bufs=2) as accp,
                tc.tile_pool(name="izpool", bufs=2) as izp,
                tc.tile_pool(name="opool", bufs=3) as op_,
                tc.tile_pool(name="psS", bufs=2, space="PSUM") as psS,
            ):
                wo_sb = wop.tile([P, G, D], BF16)        # 4 MB, resident
                for hh in range(G):
                    (nc.sync if hh % 2 == 0 else nc.gpsimd).dma_start(
                        wo_sb[:, hh], woT_t[:, hh])

                outq = [nc.sync, nc.gpsimd]
                oidx = [0]

                def attn_head(h, c_pair, ctx2, psC):
                    accs, ctxps = [], []
                    for ci, c in enumerate(c_pair):
                        acc = accp.tile([P, QC], F32R, tag=f"acc{ci}")
                        ctx_ps = psC.tile([P, QC], F32, tag=f"ctx{ci}")
                        qs = qt_sb[:, h, c * QC:(c + 1) * QC]
                        nd = 4 * c  # number of full (non-diagonal) k-tiles
                        # full tiles, two per wide PSUM tile / exp
                        for kt2 in range(0, nd, 2):
                            s2 = psS.tile([P, 2 * QC], F32, tag="s")
                            nc.tensor.matmul(
                                s2[:, 0:QC],
                                kt_sb[:, h, kt2 * P:(kt2 + 1) * P], qs,
                                start=True, stop=True)
                            nc.tensor.matmul(
                                s2[:, QC:2 * QC],
                                kt_sb[:, h, (kt2 + 1) * P:(kt2 + 2) * P], qs,
                                start=True, stop=True)
                            p2 = pp.tile([P, 2 * QC], BF16, tag="p")
                            nc.scalar.activation(p2, s2, EXP, scale=SCALE)
                            if kt2 == 0:
                                nc.vector.tensor_add(
                                    acc, p2[:, 0:QC], p2[:, QC:2 * QC])
                            else:
                                pr = prp.tile([P, QC], BF16, tag="pr")
                                nc.vector.tensor_add(
                                    pr, p2[:, 0:QC], p2[:, QC:2 * QC])
                                nc.vector.tensor_add(acc, acc, pr)
                            nc.tensor.matmul(
                                ctx_ps, vt_sb[:, kt2, h], p2[:, 0:QC],
                                start=(kt2 == 0), stop=False)
                            nc.tensor.matmul(
                                ctx_ps, vt_sb[:, kt2 + 1, h], p2[:, QC:2 * QC],
                                start=False, stop=False)
                        # diagonal pair A: j=0 at cols [0:512], j=1 at [512:896]
                        sA = psS.tile([P, 2 * QC], F32, tag="s")
                        nc.tensor.matmul(
                            sA[:, 0:QC],
                            kt_sb[:, h, nd * P:(nd + 1) * P], qs,
                            start=True, stop=True)
                        nc.tensor.matmul(
                            sA[:, QC:QC + 384],
                            kt_sb[:, h, (nd + 1) * P:(nd + 2) * P],
                            qs[:, P:QC],
                            start=True, stop=True)
                        nc.vector.tensor_add(sA[:, 0:P], sA[:, 0:P], mask_sb)
                        nc.vector.tensor_add(
                            sA[:, QC:QC + P], sA[:, QC:QC + P], mask_sb)
                        pA = pp.tile([P, 2 * QC], BF16, tag="p")
                        nc.scalar.activation(
                            pA[:, 0:QC + 384], sA[:, 0:QC + 384], EXP,
                            scale=SCALE)
                        if nd == 0:
                            nc.vector.tensor_copy(acc, pA[:, 0:QC])
                        else:
                            nc.vector.tensor_add(acc, acc, pA[:, 0:QC])
                        nc.vector.tensor_add(
                            acc[:, P:QC], acc[:, P:QC], pA[:, QC:QC + 384])
                        nc.tensor.matmul(
                            ctx_ps, vt_sb[:, nd, h], pA[:, 0:QC],
                            start=(nd == 0), stop=False)
                        nc.tensor.matmul(
                            ctx_ps[:, P:QC], vt_sb[:, nd + 1, h],
                            pA[:, QC:QC + 384],
                            start=False, stop=False)
                        # diagonal pair B: j=2 at cols [0:256], j=3 at [256:384]
                        sB = psS.tile([P, 2 * QC], F32, tag="s")
                        nc.tensor.matmul(
                            sB[:, 0:256],
                            kt_sb[:, h, (nd + 2) * P:(nd + 3) * P],
                            qs[:, 2 * P:QC],
                            start=True, stop=True)
                        nc.tensor.matmul(
                            sB[:, 256:384],
                            kt_sb[:, h, (nd + 3) * P:(nd + 4) * P],
                            qs[:, 3 * P:QC],
                            start=True, stop=True)
                        nc.vector.tensor_add(sB[:, 0:P], sB[:, 0:P], mask_sb)
                        nc.vector.tensor_add(
                            sB[:, 256:384], sB[:, 256:384], mask_sb)
                        pB = pp.tile([P, 2 * QC], BF16, tag="p")
                        nc.scalar.activation(
                            pB[:, 0:384], sB[:, 0:384], EXP, scale=SCALE)
                        nc.vector.tensor_add(
                            acc[:, 2 * P:QC], acc[:, 2 * P:QC], pB[:, 0:256])
                        nc.vector.tensor_add(
                            acc[:, 3 * P:QC], acc[:, 3 * P:QC], pB[:, 256:384])
                        nc.tensor.matmul(
                            ctx_ps[:, 2 * P:QC], vt_sb[:, nd + 2, h],
                            pB[:, 0:256],
                            start=False, stop=False)
                        nc.tensor.matmul(
                            ctx_ps[:, 3 * P:QC], vt_sb[:, nd + 3, h],
                            pB[:, 256:384],
                            start=False, stop=True)
                        accs.append(acc)
                        ctxps.append(ctx_ps)
                    # softmax denominator: broadcast partition-sum of acc
                    zw = psS.tile([P, 2 * QC], F32, tag="s")
                    nc.tensor.matmul(zw[:, 0:QC], ones_sb, accs[0],
                                     start=True, stop=True)
                    nc.tensor.matmul(zw[:, QC:2 * QC], ones_sb, accs[1],
                                     start=True, stop=True)
                    iz = izp.tile([P, 2 * QC], F32, tag="iz")
                    nc.vector.reciprocal_approx_fast(iz, zw)
                    nc.vector.tensor_mul(ctx2[:, 0, h], ctxps[0], iz[:, 0:QC])
                    nc.vector.tensor_mul(ctx2[:, 1, h], ctxps[1],
                                         iz[:, QC:2 * QC])

                def outproj_tile(nt, ci, c, ctx2src, psO):
                    o_ps = psO.tile([P, QC], F32, tag="o")
                    for hh in range(G):
                        nc.tensor.matmul(
                            o_ps, wo_sb[:, hh, nt * P:(nt + 1) * P],
                            ctx2src[:, ci, hh],
                            start=(hh == 0), stop=(hh == G - 1))
                    o_sb = op_.tile([P, QC], F32, tag="osb")
                    nc.scalar.copy(o_sb, o_ps)
                    outq[oidx[0] % 2].dma_start(
                        outT_t[:, nt, c * QC:(c + 1) * QC], o_sb)
                    oidx[0] += 1

                ctx2_0 = c2p.tile([P, 2, G, QC], BF16, tag="c2_0")
                ctx2_1 = c2p.tile([P, 2, G, QC], BF16, tag="c2_1")

                # half 0: chunks (0, 1); ctx psum double-buffered (4 banks)
                with tc.tile_pool(name="psC0", bufs=2, space="PSUM") as psC0:
                    for h in range(G):
                        attn_head(h, (0, 1), ctx2_0, psC0)

                # half 1: chunks (2, 3); interleave half-0's output projection
                with (
                    tc.tile_pool(name="psC1", bufs=1, space="PSUM") as psC1,
                    tc.tile_pool(name="psO", bufs=2, space="PSUM") as psO,
                ):
                    for h in range(G):
                        attn_head(h, (2, 3), ctx2_1, psC1)
                        for nt in (2 * h, 2 * h + 1):
                            for ci in range(2):
                                outproj_tile(nt, ci, ci, ctx2_0, psO)
                    for nt in range(NDK):
                        for ci in range(2):
                            outproj_tile(nt, ci, 2 + ci, ctx2_1, psO)

    nc.finalize()
    return nc


_NC = None


def _get_nc():
    global _NC
    if _NC is None:
        _NC = build_kernel()
    return _NC


def _make_mask():
    m = np.zeros((P, P), dtype=np.float32)
    i = np.arange(P)[:, None]
    col = np.arange(P)[None, :]
    m[i > col] = NEG
    return m


def kernel(x, Wq, Wk, Wv, Wo, _trace=False, _trace_kwargs=None):
    bf16 = ml_dtypes.bfloat16
    x = np.asarray(x, dtype=np.float32)
    Wq = np.asarray(Wq, dtype=np.float32)
    Wk = np.asarray(Wk, dtype=np.float32)
    Wv = np.asarray(Wv, dtype=np.float32)
    Wo = np.asarray(Wo, dtype=np.float32)

    nc = _get_nc()
    mask = _make_mask()

    # [d_out, d_in] -> [h, p, ko, dd] tiles per head-group chunk of 8 heads
    def tile_qk(W, g):
        wt = W.T[:, g * GD:(g + 1) * GD]              # [D, GD]
        return np.ascontiguousarray(
            wt.reshape(NDK, P, G, HD).transpose(2, 1, 0, 3).astype(bf16))

    def tile_v(W, g):
        wt = W.T[:, g * GD:(g + 1) * GD]              # [D, GD]
        return np.ascontiguousarray(
            wt.reshape(NDK, P, 2, QC).transpose(2, 1, 0, 3).astype(bf16))

    woT = np.ascontiguousarray(Wo.T)

    in_maps = []
    for core in range(8):
        b, g = divmod(core, 2)
        gs = slice(g * GD, (g + 1) * GD)
        in_maps.append({
            "xT": np.ascontiguousarray(x[b].T.astype(bf16)),
            "wqT": tile_qk(Wq, g),
            "wkT": tile_qk(Wk, g),
            "wvT": tile_v(Wv, g),
            "woT": np.ascontiguousarray(woT[gs, :].astype(bf16)),
            "maskadd": mask,
        })

    kwargs = {}
    if _trace:
        kwargs.update(trace=True, **(_trace_kwargs or {}))
    res = run_bass_kernel_spmd(nc, in_maps, core_ids=list(range(8)), **kwargs)

    out = np.empty((B, T, D), dtype=np.float32)
    for b in range(B):
        acc = res.results[2 * b]["outT"] + res.results[2 * b + 1]["outT"]
        out[b] = acc.T
    if _trace:
        return out, res
    return out
